# revision 7
# baseline (speedup 1.0000x reference)
"""BinaryConv2D forward on 8 Trainium2 NeuronCores.

out = conv2d_same(inputs, sign(clip(kernel)))   (NHWC, HWIO, 3x3, stride 1)

Sharding: data-parallel over batch (32 images -> 4 per core); the 3x3x256x256
kernel is replicated (forward only, no gradient collective needed).

Per-core kernel strategy (v2: the PE runs conv matmuls ONLY):
  - sign(w) computed on-device from bf16 cast-loads of the kernel (gpsimd
    SWDGE casts f32->bf16 in flight, one tile per cout half so the oc0
    half lands early); two Activation sign ops emit fp8e4 [cin, pair,
    cout] stationary tiles (+-1 is exact in fp8).
  - fp8 DoubleRow matmuls: one matmul contracts all 256 input channels at
    0.5 cycles/row. Precision from a two-level split x = hi + lo with
    hi = fp8(x), lo = fp8(x - hi), both streamed as accumulating passes
    (~bf16 accuracy at half the bf16 cycle count).
  - input path is PE-free: gpsimd SWDGE loads cast NHWC f32 -> bf16
    [112pix, 7blk, 256c] chunk tiles (both channel halves per DMA keeps
    the innermost run 512B, dodging the sub-512B DMA penalty); per-block
    XBAR DMA transposes (InstDmaTransposeAnt, 16x128 tiles, 98ns) on the
    SP/Act HWDGE queues emit channel-major bf16 chunk tiles; Pool derives
    hi = fp8(x), lo = fp8(x - hi) per chunk into flat-padded
    [cin, cc, 58x57] images (57-wide rows share one zero column between
    row r's x=56 pad and row r+1's x=-1 pad). Tiles are per-chunk because
    DMA writes take whole-tile WAR deps against earlier readers — shared
    tiles would serialize the chunk pipeline.
  - conv as 9 shifted flat-window DoubleRow matmuls x {hi,lo} per psum
    block of 8 output rows (N=456, pad-row slices clipped at the image
    edges), accumulating 18 matmuls.
  - output path: DVE evicts PSUM -> bf16 [cout, pix]; after the oc's last
    evict, 25 XBAR transposes flip 128-pixel blocks back to pixel-major
    (emitted only after all evicts so the whole-tile WAR never stalls an
    evict), DVE upcasts bf16 -> f32, natural-layout DMA stores. All of it
    overlaps the next oc's conv. The very last oc instead uses v1's PE
    transpose path (PE is free at the tail) with fine-grained per-group
    stores, keeping the post-conv drain short.
  - warmup matmuls at t=0 keep the PE p-state ramp clock running while
    image 0 loads; image 0's hi/lo runs on DVE (idle at startup) except
    chunk 1 (Pool); steady-state images prefetch on Pool/SP/Act entirely
    under the previous image's conv.

Cost-model (CoreSim) lineage: 450.4 us (bf16 2-pass) -> 141.8 us (v1:
fp8 DR + PE transposes) -> this rewrite (PE-only-conv + XBAR transposes).
"""

import numpy as np

P = 128
H = 56
W = 56
C = 256
XW = W + 2                   # padded row count (58: rows y=-1..56)
RW = W + 1                   # flat row stride: one shared zero col per row
FL = XW * RW                 # flat padded image length (3306)
FT = 3312                    # fp8 tile free size (junk pad to %16)
NCORES = 8
NTOT = 32
NI = NTOT // NCORES          # images per core
NPIX = H * W                 # 3136
RB = 8                       # output rows per psum block
NT = H // RB                 # 7 psum blocks
TB = 112                     # pixels per transpose block (= 2 rows)
NBLK = NPIX // TB            # 28 blocks exactly
NCH = 4                      # load chunks per image (7 blocks each)
CB = 7 * TB                  # pixels per chunk (784)
OB = 128                     # output transpose block (XBAR needs %128)
NOB = 25                     # ceil(3136/128) output blocks (last is 64 real)
OPIX = OB * NOB              # 3200 padded output pixels

_cache = {}


def _build_bass(ni=NI, loops=1, warm=76):
    import concourse.bacc as bacc
    import concourse.mybir as mybir
    import concourse.tile as tile
    from concourse.masks import make_identity
    from contextlib import ExitStack

    f32 = mybir.dt.float32
    bf16 = mybir.dt.bfloat16
    fp8 = mybir.dt.float8e4
    DR = mybir.MatmulPerfMode.DoubleRow

    nc = bacc.Bacc()
    x = nc.dram_tensor("x", [ni, NPIX, C], f32, kind="ExternalInput")
    w = nc.dram_tensor("w", [3, 3, C, C], f32, kind="ExternalInput")
    y = nc.dram_tensor("y", [ni, NPIX, C], f32, kind="ExternalOutput")

    with ExitStack() as ctx:
        tc = ctx.enter_context(tile.TileContext(nc))
        const = ctx.enter_context(tc.tile_pool(name="const", bufs=1))
        wpool = ctx.enter_context(tc.tile_pool(name="wpool", bufs=1))
        wstage = ctx.enter_context(tc.tile_pool(name="wstage", bufs=1))
        hinp = ctx.enter_context(tc.tile_pool(name="hinp", bufs=2))
        xbp = ctx.enter_context(tc.tile_pool(name="xbp", bufs=2))
        padp = ctx.enter_context(tc.tile_pool(name="padp", bufs=2))
        ocp = ctx.enter_context(tc.tile_pool(name="ocp", bufs=2))
        otp = ctx.enter_context(tc.tile_pool(name="otp", bufs=2))
        onp = ctx.enter_context(tc.tile_pool(name="onp", bufs=2))
        onf = ctx.enter_context(tc.tile_pool(name="onf", bufs=2))
        psc = ctx.enter_context(tc.tile_pool(name="psc", bufs=3, space="PSUM"))
        psf = ctx.enter_context(tc.tile_pool(name="psf", bufs=2, space="PSUM"))

        identb = const.tile([P, P], bf16)
        make_identity(nc, identb)

        # ---- binarized weight tiles: sign(w) as fp8 [cin, pair, cout],
        # one bf16 stage tile per cout half (keeps each SWDGE load's
        # innermost run 512B and lets the oc0 half land early) ----
        wst = [wstage.tile([P, 9, 2, P], bf16, name=f"wst{o}")
               for o in range(2)]
        wsgn = wpool.tile([P, 9, 2, 2, P], fp8, name="wsgn")

        def _load_w(oc):
            nc.gpsimd.dma_start(
                out=wst[oc],
                in_=w[:, :, :, P * oc : P * (oc + 1)].rearrange(
                    "ky kx (cc p) o -> p (ky kx) cc o", p=P
                ),
            )

        def _sign(oc):
            nc.scalar.sign(out=wsgn[:, :, :, oc, :], in_=wst[oc])

        # HAM warmup: dummy matmuls keep the PE busy from t~0 while the
        # first image loads, so the p-state ramp reaches full clock before
        # the first conv matmuls arrive. Results are never read.
        wrm = psc.tile([P, RB, RW], f32, name="ps")
        for _ in range(warm):
            nc.tensor.matmul(
                wrm[:, :2, :], lhsT=identb, rhs=identb[:, : 2 * RW],
                start=True, stop=True,
            )

        dmaq = [nc.sync, nc.scalar]

        def _alloc_image(img):
            st = {"img": img}
            st["hin"] = [hinp.tile([TB, 7, C], bf16, name=f"hin{q}")
                         for q in range(NCH)]
            st["xpb"] = [xbp.tile([P, 2, CB], bf16, name=f"xpb{q}")
                         for q in range(NCH)]
            st["xph"] = padp.tile([P, 2, FT], fp8, name="xph")
            st["xpl"] = padp.tile([P, 2, FT], fp8, name="xpl")
            # zero the SAME-padding borders (rows y=-1,56 and cols x=-1,56)
            # and the junk edge cells some shifted windows read
            for xp8 in (st["xph"], st["xpl"]):
                nc.vector.memset(xp8[:, :, 0:1], 0.0)
                nc.vector.memset(xp8[:, :, 1 + FL : FT], 0.0)
                xv = xp8[:, :, 1 : 1 + FL].rearrange(
                    "p j (r c) -> p j r c", c=RW
                )
                nc.vector.memset(xv[:, :, 0, :], 0.0)
                nc.vector.memset(xv[:, :, XW - 1, :], 0.0)
                nc.vector.memset(xv[:, :, 1 : XW - 1, 0], 0.0)
            return st

        def _load_chunk(st, q):
            # gpsimd SWDGE load, casting f32 -> bf16 in flight; both channel
            # halves in one DMA (innermost 512B out keeps full DMA rate)
            nc.gpsimd.dma_start(
                out=st["hin"][q],
                in_=x[st["img"], :, :].rearrange(
                    "(q b p) c -> q p b c", p=TB, q=NCH
                )[q],
            )

        def _emit_xbar(st, q, queues=(0, 1)):
            # XBAR-transpose chunk q's blocks into channel-major bf16
            for b in range(7):
                for cc in range(2):
                    dmaq[queues[(b + cc) % len(queues)]].dma_start(
                        out=st["xpb"][q][:, cc, TB * b : TB * (b + 1)],
                        in_=st["hin"][q][:, b, P * cc : P * (cc + 1)],
                        transpose=True,
                    )

        def _emit_hilo(st, q, eng):
            # derive hi = fp8(x), lo = fp8(x - hi) for chunk q (14 image
            # rows) into the flat-padded fp8 images
            r0 = 14 * q + 1
            for cc in range(2):
                bv = st["xpb"][q][:, cc, :].rearrange(
                    "p (r c) -> p r c", c=W
                )

                def _dst(xp8):
                    return xp8[:, cc, 1 : 1 + FL].rearrange(
                        "p (r c) -> p r c", c=RW
                    )[:, r0 : r0 + 14, 1 : 1 + W]

                eng.tensor_copy(out=_dst(st["xph"]), in_=bv)
                eng.tensor_sub(out=_dst(st["xpl"]), in0=bv,
                               in1=_dst(st["xph"]))

        def _emit_out_xbar(img, oc, ocmp):
            # flip the finished oc image back to pixel-major, upcast, store.
            # Runs entirely on SP/Act/DVE, overlapping the next oc's conv.
            ot = otp.tile([P, NOB, P], bf16, name="ot")
            onat = onp.tile([P, NOB, P], f32, name="onat")
            for j in range(NOB):
                dmaq[j % 2].dma_start(
                    out=ot[:, j, :],
                    in_=ocmp[:, OB * j : OB * (j + 1)],
                    transpose=True,
                )
            for bi, (j0, j1) in enumerate(((0, 12), (12, NOB))):
                nc.vector.tensor_copy(out=onat[:, j0:j1], in_=ot[:, j0:j1])
                jr = min(j1, NOB - 1)
                dmaq[bi].dma_start(
                    out=y[
                        img, OB * j0 : OB * jr, P * oc : P * (oc + 1)
                    ].rearrange("(b p) c -> p b c", p=OB),
                    in_=onat[:, j0:jr],
                )
            dmaq[0].dma_start(
                out=y[img, OB * (NOB - 1) : NPIX, P * oc : P * (oc + 1)
                      ].rearrange("(b p) c -> p b c", p=64),
                in_=onat[:64, NOB - 1],
            )

        def _emit_group_pe(img, oc, ocmp, t):
            # v1-style PE transpose path for the very last oc: group t's 4
            # TB-blocks (448 pixels) go psum->sbuf->store right away, so
            # the post-conv drain is one small group
            pt = psf.tile([P, 4, P], bf16, name="ptf")
            onatf = onf.tile([P, 4, P], f32, name="onatf")
            for bi in range(4):
                b = 4 * t + bi
                nc.tensor.transpose(
                    pt[:TB, bi, :], ocmp[:, TB * b : TB * (b + 1)], identb
                )
            nc.scalar.copy(out=onatf[:TB, :], in_=pt[:TB, :])
            for qi in range(2):
                dmaq[qi].dma_start(
                    out=y[
                        img,
                        TB * (4 * t + 2 * qi) : TB * (4 * t + 2 * qi + 2),
                        P * oc : P * (oc + 1),
                    ].rearrange("(b p) c -> p b c", p=TB),
                    in_=onatf[:TB, 2 * qi : 2 * qi + 2],
                )

        def _conv_image(st, nxt):
            # ---- conv: 18 accumulating DoubleRow matmuls per psum block
            # (hi/lo passes x 9 taps, all 256 cin per matmul). All other
            # work rides on DVE/Pool/SP/Act and never touches the PE queue
            # (except the last oc's tail transposes, when the PE is free).
            img = st["img"]
            combos = [
                (st["xph"], ky, kx) for ky in (1, 0, 2) for kx in range(3)
            ] + [
                (st["xpl"], ky, kx) for ky in (1, 0, 2) for kx in range(3)
            ]
            n_c = len(combos)

            for oc in range(2):
                fine = nxt is None and oc == 1
                ocmp = ocp.tile([P, OPIX], bf16, name="ocmp")
                if not fine:
                    nc.vector.memset(ocmp[:, NPIX:OPIX], 0.0)
                for t in range(NT):
                    ps = psc.tile([P, RB, RW], f32, name="ps")
                    for ci, (src8, ky, kx) in enumerate(combos):
                        dy = ky - 1
                        fs = (RB * t + dy + 1) * RW + kx
                        # skip the zero pad-row slice of the window for the
                        # edge taps (ci==0 is dy=0, so the start flag still
                        # clears the full region)
                        r0 = 1 if (t == 0 and dy < 0) else 0
                        r1 = RB - (1 if (t == NT - 1 and dy > 0) else 0)
                        nc.tensor.matmul(
                            ps[:, r0:r1, :],
                            lhsT=wsgn[:, 3 * ky + kx, :, oc, :],
                            rhs=src8[:, :, fs + r0 * RW : fs + r1 * RW],
                            start=(ci == 0),
                            stop=(ci == n_c - 1),
                            perf_mode=DR,
                        )
                        # spread next-image prep between this group's
                        # matmuls (non-PE queues, dependency-time order)
                        if nxt is not None and ci == 4:
                            if oc == 0:
                                if t == 0:
                                    for q in range(NCH):
                                        _load_chunk(nxt, q)
                                elif t in (1, 2, 3, 4):
                                    _emit_xbar(nxt, t - 1)
                            elif t in (0, 1, 2, 3):
                                _emit_hilo(nxt, t, nc.gpsimd)
                    # evict this group's rows to the bf16 compact image
                    nc.vector.tensor_copy(
                        out=ocmp[:, RB * W * t : RB * W * (t + 1)],
                        in_=ps[:, :, 1 : 1 + W],
                    )
                    if fine:
                        _emit_group_pe(img, oc, ocmp, t)
                if not fine:
                    _emit_out_xbar(img, oc, ocmp)

        def _images():
            # image 0 startup: chunk-0's load -> XBAR -> hi/lo chain is the
            # critical path to the first conv matmul. Pool queue order puts
            # c0 first, the oc0 weight half second; image-0 hi/lo runs on
            # DVE (idle at startup) except chunk 1 (Pool, between loads);
            # chunk 0's XBARs run on SP alone so the Act queue's sign0
            # never delays them.
            st = _alloc_image(0)
            _load_chunk(st, 0)
            _load_w(0)
            _sign(0)
            _emit_xbar(st, 0, queues=(0,))
            _emit_hilo(st, 0, nc.vector)
            _load_chunk(st, 1)
            _emit_xbar(st, 1, queues=(1,))
            _emit_hilo(st, 1, nc.gpsimd)
            _load_chunk(st, 2)
            _emit_xbar(st, 2)
            _emit_hilo(st, 2, nc.vector)
            _load_chunk(st, 3)
            _load_w(1)
            _emit_xbar(st, 3)
            _sign(1)
            _emit_hilo(st, 3, nc.vector)
            for img in range(ni):
                nxt = _alloc_image(img + 1) if img + 1 < ni else None
                _conv_image(st, nxt)
                st = nxt

        if loops == 1:
            _images()
        else:
            with tc.For_i(0, loops, 1):
                _images()
    nc.compile()
    return nc


def get_bass(ni=NI, loops=1):
    key = (ni, loops)
    if key not in _cache:
        _cache[key] = _build_bass(ni, loops)
    return _cache[key]


def run(inputs, kernel, trace=False, **kw):
    from concourse.bass_utils import run_bass_kernel_spmd

    nc = get_bass()
    xs = np.ascontiguousarray(inputs, dtype=np.float32).reshape(NTOT, NPIX, C)
    wf = np.ascontiguousarray(kernel, dtype=np.float32)
    in_maps = [
        {"x": xs[i * NI : (i + 1) * NI], "w": wf} for i in range(NCORES)
    ]
    res = run_bass_kernel_spmd(nc, in_maps, core_ids=list(range(NCORES)),
                               trace=trace, **kw)
    out = np.concatenate([r["y"] for r in res.results], axis=0)
    return out.reshape(NTOT, H, W, C), res


def kernel(**inputs):
    out, _ = run(inputs["inputs"], inputs["kernel"])
    return out


# revision 12
# speedup vs baseline: 1.1270x; 1.1270x over previous
"""BinaryConv2D forward on 8 Trainium2 NeuronCores.

out = conv2d_same(inputs, sign(clip(kernel)))   (NHWC, HWIO, 3x3, stride 1)

Sharding: data-parallel over batch (32 images -> 4 per core); the 3x3x256x256
kernel is replicated (forward only, no gradient collective needed).

Per-core kernel strategy (v2: the PE runs conv matmuls ONLY):
  - weights: two f32 HWDGE loads (one per cout half, parallel on the
    SP/Act queues) feed two Activation sign ops emitting fp8e4
    [cin, pair, cout] stationary tiles (+-1 is exact in fp8), one tile
    per cout half so oc0's matmuls never wait on oc1's sign.
  - fp8 DoubleRow matmuls: one matmul contracts all 256 input channels at
    0.5 cycles/row. Precision from a two-level split x = hi + lo with
    hi = fp8(x), lo = fp8(x - hi), both streamed as accumulating passes
    (~bf16 accuracy at half the bf16 cycle count).
  - input path is PE-free: gpsimd SWDGE loads cast NHWC f32 -> bf16
    [112pix, 7blk, 256c] chunk tiles (both channel halves per DMA keeps
    the innermost run 512B, dodging the sub-512B DMA penalty; a 128KB
    descriptor carveout keeps the SWDGE FIFO from serializing);
    per-block XBAR DMA transposes (InstDmaTransposeAnt, 16x128 tiles,
    98ns) on the SP/Act HWDGE queues emit channel-major bf16 chunk
    tiles; Pool derives hi = fp8(x), lo = fp8(x - hi) per chunk into
    flat-padded [cin, cc, 58x57] images (57-wide rows share one zero
    column between row r's x=56 pad and row r+1's x=-1 pad). Tiles are
    per-chunk because DMA writes take whole-tile WAR deps against
    earlier readers — shared tiles would serialize the chunk pipeline.
  - conv as 9 shifted flat-window DoubleRow matmuls x {hi,lo} per psum
    block of 8 output rows (N=456, pad-row slices clipped at the image
    edges), accumulating 18 matmuls.
  - output path: DVE evicts PSUM -> bf16 [cout, pix]; after the oc's last
    evict, 25 XBAR transposes flip 128-pixel blocks back to pixel-major
    (emitted only after all evicts so the whole-tile WAR never stalls an
    evict), DVE upcasts bf16 -> f32, natural-layout DMA stores. All of it
    overlaps the next oc's conv. The very last oc instead uses v1's PE
    transpose path (PE is free at the tail) with fine-grained per-group
    stores, keeping the post-conv drain short.
  - warmup matmuls at t=0 keep the PE p-state ramp clock running while
    image 0 loads; image 0's hi/lo runs on DVE (chunks 0-1) and Pool
    (chunks 2-3); steady-state images prefetch on Pool/SP/Act entirely
    under the previous image's conv.

Cost-model (CoreSim) lineage: 450.4 us (bf16 2-pass) -> 141.8 us (v1:
fp8 DR + PE transposes) -> this rewrite (PE-only-conv + XBAR transposes).
"""

import numpy as np

P = 128
H = 56
W = 56
C = 256
XW = W + 2                   # padded row count (58: rows y=-1..56)
RW = W + 1                   # flat row stride: one shared zero col per row
FL = XW * RW                 # flat padded image length (3306)
FT = 3312                    # fp8 tile free size (junk pad to %16)
NCORES = 8
NTOT = 32
NI = NTOT // NCORES          # images per core
NPIX = H * W                 # 3136
RB = 8                       # output rows per psum block
NT = H // RB                 # 7 psum blocks
TB = 112                     # pixels per transpose block (= 2 rows)
NBLK = NPIX // TB            # 28 blocks exactly
NCH = 4                      # load chunks per image (7 blocks each)
CB = 7 * TB                  # pixels per chunk (784)
OB = 128                     # output transpose block (XBAR needs %128)
NOB = 25                     # ceil(3136/128) output blocks (last is 64 real)
OPIX = OB * NOB              # 3200 padded output pixels

_cache = {}


def _build_bass(ni=NI, loops=1, warm=135):
    import concourse.bacc as bacc
    import concourse.mybir as mybir
    import concourse.tile as tile
    from concourse.masks import make_identity
    from contextlib import ExitStack

    f32 = mybir.dt.float32
    bf16 = mybir.dt.bfloat16
    fp8 = mybir.dt.float8e4
    DR = mybir.MatmulPerfMode.DoubleRow

    nc = bacc.Bacc(dynamic_dma_scratch_size=49152)
    x = nc.dram_tensor("x", [ni, NPIX, C], f32, kind="ExternalInput")
    w = nc.dram_tensor("w", [3, 3, C, C], f32, kind="ExternalInput")
    y = nc.dram_tensor("y", [ni, NPIX, C], f32, kind="ExternalOutput")

    with ExitStack() as ctx:
        tc = ctx.enter_context(tile.TileContext(nc))
        const = ctx.enter_context(tc.tile_pool(name="const", bufs=1))
        wpool = ctx.enter_context(tc.tile_pool(name="wpool", bufs=1))
        wstage = ctx.enter_context(tc.tile_pool(name="wstage", bufs=1))
        hinp = ctx.enter_context(tc.tile_pool(name="hinp", bufs=1))
        xbp = ctx.enter_context(tc.tile_pool(name="xbp", bufs=1))
        padp = ctx.enter_context(tc.tile_pool(name="padp", bufs=2))
        ocp = ctx.enter_context(tc.tile_pool(name="ocp", bufs=2))
        otp = ctx.enter_context(tc.tile_pool(name="otp", bufs=2))
        onp = ctx.enter_context(tc.tile_pool(name="onp", bufs=2))
        onf = ctx.enter_context(tc.tile_pool(name="onf", bufs=2))
        psc = ctx.enter_context(tc.tile_pool(name="psc", bufs=3, space="PSUM"))
        psf = ctx.enter_context(tc.tile_pool(name="psf", bufs=2, space="PSUM"))

        identb = const.tile([P, P], bf16)
        make_identity(nc, identb)

        dmaq = [nc.sync, nc.scalar]

        # ---- binarized weight tiles: sign(w) as fp8 [cin, pair, cout],
        # one shared f32 stage tile (reloaded per cout half) + fp8 sign
        # tiles per half so oc0's matmuls never wait on oc1's sign ----
        wsgn = [wpool.tile([P, 9, 2, P], fp8, name=f"wsgn{o}")
                for o in range(2)]

        def _load_w(oc):
            wt = wstage.tile([P, 9, 2, P], f32, name="wst")
            dmaq[oc].dma_start(
                out=wt,
                in_=w[:, :, :, P * oc : P * (oc + 1)].rearrange(
                    "ky kx (cc p) o -> p (ky kx) cc o", p=P
                ),
            )
            return wt

        def _sign(oc, wt):
            nc.scalar.sign(out=wsgn[oc], in_=wt)

        # HAM warmup: dummy matmuls keep the PE busy from t~0 while the
        # first image loads, so the p-state ramp reaches full clock before
        # the first conv matmuls arrive. Results are never read.
        wrm = psc.tile([P, RB, RW], f32, name="ps")
        for _ in range(warm):
            nc.tensor.matmul(
                wrm[:, :2, :], lhsT=identb, rhs=identb[:, : 2 * RW],
                start=True, stop=True,
            )

        def _alloc_image(img):
            st = {"img": img}
            st["hin"] = [hinp.tile([TB, 7, C], bf16, name=f"hin{q}")
                         for q in range(NCH)]
            st["xpb"] = [xbp.tile([P, 2, CB], bf16, name=f"xpb{q}")
                         for q in range(NCH)]
            st["xph"] = padp.tile([P, 2, FT], fp8, name="xph")
            st["xpl"] = padp.tile([P, 2, FT], fp8, name="xpl")
            # zero the SAME-padding borders (rows y=-1,56 and cols x=-1,56)
            # and the junk edge cells some shifted windows read
            for xp8 in (st["xph"], st["xpl"]):
                nc.vector.memset(xp8[:, :, 0:1], 0.0)
                nc.vector.memset(xp8[:, :, 1 + FL : FT], 0.0)
                xv = xp8[:, :, 1 : 1 + FL].rearrange(
                    "p j (r c) -> p j r c", c=RW
                )
                nc.vector.memset(xv[:, :, 0, :], 0.0)
                nc.vector.memset(xv[:, :, XW - 1, :], 0.0)
                nc.vector.memset(xv[:, :, 1 : XW - 1, 0], 0.0)
            return st

        def _load_chunk(st, q):
            # gpsimd SWDGE load, casting f32 -> bf16 in flight; both channel
            # halves in one DMA (innermost 512B out keeps full DMA rate)
            nc.gpsimd.dma_start(
                out=st["hin"][q],
                in_=x[st["img"], :, :].rearrange(
                    "(q b p) c -> q p b c", p=TB, q=NCH
                )[q],
            )

        def _emit_xbar(st, q, queues=(0, 1)):
            # XBAR-transpose chunk q's blocks into channel-major bf16
            for b in range(7):
                for cc in range(2):
                    dmaq[queues[(b + cc) % len(queues)]].dma_start(
                        out=st["xpb"][q][:, cc, TB * b : TB * (b + 1)],
                        in_=st["hin"][q][:, b, P * cc : P * (cc + 1)],
                        transpose=True,
                    )

        def _emit_hilo(st, q, eng):
            # derive hi = fp8(x), lo = fp8(x - hi) for chunk q (14 image
            # rows) into the flat-padded fp8 images
            r0 = 14 * q + 1
            for cc in range(2):
                bv = st["xpb"][q][:, cc, :].rearrange(
                    "p (r c) -> p r c", c=W
                )

                def _dst(xp8):
                    return xp8[:, cc, 1 : 1 + FL].rearrange(
                        "p (r c) -> p r c", c=RW
                    )[:, r0 : r0 + 14, 1 : 1 + W]

                eng.tensor_copy(out=_dst(st["xph"]), in_=bv)
                eng.tensor_sub(out=_dst(st["xpl"]), in0=bv,
                               in1=_dst(st["xph"]))

        def _emit_out_xbar(img, oc, ocmp):
            # flip the finished oc image back to pixel-major, upcast, store.
            # Runs entirely on SP/Act/DVE, overlapping the next oc's conv.
            ot = otp.tile([P, NOB, P], bf16, name="ot")
            onat = onp.tile([P, NOB, P], f32, name="onat")
            for j in range(NOB):
                dmaq[(j + oc) % 2].dma_start(
                    out=ot[:, j, :],
                    in_=ocmp[:, OB * j : OB * (j + 1)],
                    transpose=True,
                )
            for bi, (j0, j1) in enumerate(((0, 12), (12, NOB))):
                nc.vector.tensor_copy(out=onat[:, j0:j1], in_=ot[:, j0:j1])
                jr = min(j1, NOB - 1)
                dmaq[(bi + oc) % 2].dma_start(
                    out=y[
                        img, OB * j0 : OB * jr, P * oc : P * (oc + 1)
                    ].rearrange("(b p) c -> p b c", p=OB),
                    in_=onat[:, j0:jr],
                )
            dmaq[oc % 2].dma_start(
                out=y[img, OB * (NOB - 1) : NPIX, P * oc : P * (oc + 1)
                      ].rearrange("(b p) c -> p b c", p=64),
                in_=onat[:64, NOB - 1],
            )

        def _emit_group_pe(img, oc, ocmp, t):
            # v1-style PE transpose path for the very last oc: group t's 4
            # TB-blocks (448 pixels) go psum->sbuf->store right away, so
            # the post-conv drain is one small group
            pt = psf.tile([P, 4, P], bf16, name="ptf")
            onatf = onf.tile([P, 4, P], f32, name="onatf")
            for bi in range(4):
                b = 4 * t + bi
                nc.tensor.transpose(
                    pt[:TB, bi, :], ocmp[:, TB * b : TB * (b + 1)], identb
                )
            nc.scalar.copy(out=onatf[:TB, :], in_=pt[:TB, :])
            for qi in range(2):
                dmaq[qi].dma_start(
                    out=y[
                        img,
                        TB * (4 * t + 2 * qi) : TB * (4 * t + 2 * qi + 2),
                        P * oc : P * (oc + 1),
                    ].rearrange("(b p) c -> p b c", p=TB),
                    in_=onatf[:TB, 2 * qi : 2 * qi + 2],
                )

        def _conv_image(st, nxt):
            # ---- conv: 18 accumulating DoubleRow matmuls per psum block
            # (hi/lo passes x 9 taps, all 256 cin per matmul). All other
            # work rides on DVE/Pool/SP/Act and never touches the PE queue
            # (except the last oc's tail transposes, when the PE is free).
            img = st["img"]
            combos = [
                (st["xph"], ky, kx) for ky in (1, 0, 2) for kx in range(3)
            ] + [
                (st["xpl"], ky, kx) for ky in (1, 0, 2) for kx in range(3)
            ]
            n_c = len(combos)

            for oc in range(2):
                fine = nxt is None and oc == 1
                ocmp = ocp.tile([P, OPIX], bf16, name="ocmp")
                if not fine:
                    nc.vector.memset(ocmp[:, NPIX:OPIX], 0.0)
                for t in range(NT):
                    ps = psc.tile([P, RB, RW], f32, name="ps")
                    for ci, (src8, ky, kx) in enumerate(combos):
                        dy = ky - 1
                        fs = (RB * t + dy + 1) * RW + kx
                        # skip the zero pad-row slice of the window for the
                        # edge taps (ci==0 is dy=0, so the start flag still
                        # clears the full region)
                        r0 = 1 if (t == 0 and dy < 0) else 0
                        r1 = RB - (1 if (t == NT - 1 and dy > 0) else 0)
                        nc.tensor.matmul(
                            ps[:, r0:r1, :],
                            lhsT=wsgn[oc][:, 3 * ky + kx, :, :],
                            rhs=src8[:, :, fs + r0 * RW : fs + r1 * RW],
                            start=(ci == 0),
                            stop=(ci == n_c - 1),
                            perf_mode=DR,
                        )
                        # spread next-image prep between this group's
                        # matmuls (non-PE queues, dependency-time order)
                        if nxt is not None and ci == 4:
                            if oc == 0:
                                if t == 0:
                                    for q in range(NCH):
                                        _load_chunk(nxt, q)
                                elif t in (1, 2, 3, 4):
                                    _emit_xbar(nxt, t - 1)
                            elif t in (0, 1, 2, 3):
                                _emit_hilo(nxt, t, nc.gpsimd)
                    # evict this group's rows to the bf16 compact image
                    nc.vector.tensor_copy(
                        out=ocmp[:, RB * W * t : RB * W * (t + 1)],
                        in_=ps[:, :, 1 : 1 + W],
                    )
                    if fine:
                        _emit_group_pe(img, oc, ocmp, t)
                if not fine:
                    _emit_out_xbar(img, oc, ocmp)

        def _images():
            # image 0 startup: chunk-0's load -> XBAR -> hi/lo chain is the
            # critical path to the first conv matmul. Weights ride the
            # HWDGE queues (parallel to the SWDGE chunk loads on Pool);
            # image-0 hi/lo runs on DVE (chunks 0-1) and Pool (2-3).
            st = _alloc_image(0)
            _load_chunk(st, 0)
            wt0 = _load_w(0)
            _load_chunk(st, 1)
            _load_chunk(st, 2)
            _load_chunk(st, 3)
            _emit_xbar(st, 0)
            _sign(0, wt0)
            _emit_hilo(st, 0, nc.vector)
            _emit_xbar(st, 1, queues=(0,))
            _emit_hilo(st, 1, nc.vector)
            _emit_xbar(st, 2, queues=(0,))
            wt1 = _load_w(1)
            _sign(1, wt1)
            _emit_hilo(st, 2, nc.gpsimd)
            _emit_xbar(st, 3, queues=(0,))
            _emit_hilo(st, 3, nc.gpsimd)
            for img in range(ni):
                nxt = _alloc_image(img + 1) if img + 1 < ni else None
                _conv_image(st, nxt)
                st = nxt

        if loops == 1:
            _images()
        else:
            with tc.For_i(0, loops, 1):
                _images()
    nc.compile()
    return nc


def get_bass(ni=NI, loops=1):
    key = (ni, loops)
    if key not in _cache:
        _cache[key] = _build_bass(ni, loops)
    return _cache[key]


def run(inputs, kernel, trace=False, **kw):
    from concourse.bass_utils import run_bass_kernel_spmd

    nc = get_bass()
    xs = np.ascontiguousarray(inputs, dtype=np.float32).reshape(NTOT, NPIX, C)
    wf = np.ascontiguousarray(kernel, dtype=np.float32)
    in_maps = [
        {"x": xs[i * NI : (i + 1) * NI], "w": wf} for i in range(NCORES)
    ]
    res = run_bass_kernel_spmd(nc, in_maps, core_ids=list(range(NCORES)),
                               trace=trace, **kw)
    out = np.concatenate([r["y"] for r in res.results], axis=0)
    return out.reshape(NTOT, H, W, C), res


def kernel(**inputs):
    out, _ = run(inputs["inputs"], inputs["kernel"])
    return out


# revision 13
# speedup vs baseline: 1.3447x; 1.1931x over previous
"""BinaryConv2D forward on 8 Trainium2 NeuronCores.

out = conv2d_same(inputs, sign(clip(kernel)))   (NHWC, HWIO, 3x3, stride 1)

Sharding: data-parallel over batch (32 images -> 4 per core); the 3x3x256x256
kernel is replicated (forward only, no gradient collective needed).

Per-core kernel strategy (v2: the PE runs conv matmuls ONLY):
  - weights: two f32 HWDGE loads (one per cout half, parallel on the
    SP/Act queues) feed two Activation sign ops emitting fp8e4
    [cin, pair, cout] stationary tiles (+-1 is exact in fp8), one tile
    per cout half so oc0's matmuls never wait on oc1's sign.
  - fp8 DoubleRow matmuls: one matmul contracts all 256 input channels at
    0.5 cycles/row. Precision from a two-level split x = hi + lo with
    hi = fp8(x), lo = fp8(x - hi), both streamed as accumulating passes
    (~bf16 accuracy at half the bf16 cycle count).
  - input path is PE-free: gpsimd SWDGE loads cast NHWC f32 -> bf16
    [112pix, 7blk, 256c] chunk tiles (both channel halves per DMA keeps
    the innermost run 512B, dodging the sub-512B DMA penalty; a 128KB
    descriptor carveout keeps the SWDGE FIFO from serializing);
    per-block XBAR DMA transposes (InstDmaTransposeAnt, 16x128 tiles,
    98ns) on the SP/Act HWDGE queues emit channel-major bf16 chunk
    tiles; Pool derives hi = fp8(x), lo = fp8(x - hi) per chunk into
    flat-padded [cin, cc, 58x57] images (57-wide rows share one zero
    column between row r's x=56 pad and row r+1's x=-1 pad). Tiles are
    per-chunk because DMA writes take whole-tile WAR deps against
    earlier readers — shared tiles would serialize the chunk pipeline.
  - conv as 9 shifted flat-window DoubleRow matmuls x {hi,lo} per psum
    block of 8 output rows (N=456, pad-row slices clipped at the image
    edges), accumulating 18 matmuls.
  - output path: DVE evicts PSUM -> bf16 [cout, pix]; after the oc's last
    evict, 25 XBAR transposes flip 128-pixel blocks back to pixel-major
    (emitted only after all evicts so the whole-tile WAR never stalls an
    evict), DVE upcasts bf16 -> f32, natural-layout DMA stores. All of it
    overlaps the next oc's conv. The very last oc instead uses v1's PE
    transpose path (PE is free at the tail) with fine-grained per-group
    stores, keeping the post-conv drain short.
  - warmup matmuls at t=0 keep the PE p-state ramp clock running while
    image 0 loads; image 0's hi/lo runs on DVE (chunks 0-1) and Pool
    (chunks 2-3); steady-state images prefetch on Pool/SP/Act entirely
    under the previous image's conv.

Cost-model (CoreSim) lineage: 450.4 us (bf16 2-pass) -> 141.8 us (v1:
fp8 DR + PE transposes) -> this rewrite (PE-only-conv + XBAR transposes).
"""

import numpy as np

P = 128
H = 56
W = 56
C = 256
XW = W + 2                   # padded row count (58: rows y=-1..56)
RW = W + 1                   # flat row stride: one shared zero col per row
FL = XW * RW                 # flat padded image length (3306)
FT = 3312                    # fp8 tile free size (junk pad to %16)
NCORES = 8
NTOT = 32
NI = NTOT // NCORES          # images per core
NPIX = H * W                 # 3136
RB = 8                       # output rows per psum block
NT = H // RB                 # 7 psum blocks
TB = 112                     # pixels per transpose block (= 2 rows)
NBLK = NPIX // TB            # 28 blocks exactly
NCH = 4                      # load chunks per image (7 blocks each)
CB = 7 * TB                  # pixels per chunk (784)
OB = 128                     # output transpose block (XBAR needs %128)
NOB = 25                     # ceil(3136/128) output blocks (last is 64 real)
OPIX = OB * NOB              # 3200 padded output pixels

_cache = {}


def _build_bass(ni=NI, loops=1, warm=135):
    import concourse.bacc as bacc
    import concourse.mybir as mybir
    import concourse.tile as tile
    from concourse.masks import make_identity
    from contextlib import ExitStack

    f32 = mybir.dt.float32
    bf16 = mybir.dt.bfloat16
    fp8 = mybir.dt.float8e4
    DR = mybir.MatmulPerfMode.DoubleRow

    nc = bacc.Bacc(dynamic_dma_scratch_size=81920)
    x = nc.dram_tensor("x", [ni, NPIX, C], f32, kind="ExternalInput")
    w = nc.dram_tensor("w", [3, 3, C, C], f32, kind="ExternalInput")
    y = nc.dram_tensor("y", [ni, NPIX, C], f32, kind="ExternalOutput")

    with ExitStack() as ctx:
        tc = ctx.enter_context(tile.TileContext(nc))
        const = ctx.enter_context(tc.tile_pool(name="const", bufs=1))
        wpool = ctx.enter_context(tc.tile_pool(name="wpool", bufs=1))
        wstage = ctx.enter_context(tc.tile_pool(name="wstage", bufs=1))
        hinp = ctx.enter_context(tc.tile_pool(name="hinp", bufs=1))
        xbp = ctx.enter_context(tc.tile_pool(name="xbp", bufs=1))
        padp = ctx.enter_context(tc.tile_pool(name="padp", bufs=2))
        ocp = ctx.enter_context(tc.tile_pool(name="ocp", bufs=2))
        otp = ctx.enter_context(tc.tile_pool(name="otp", bufs=2))
        onp = ctx.enter_context(tc.tile_pool(name="onp", bufs=2))
        onf = ctx.enter_context(tc.tile_pool(name="onf", bufs=2))
        psc = ctx.enter_context(tc.tile_pool(name="psc", bufs=3, space="PSUM"))
        psf = ctx.enter_context(tc.tile_pool(name="psf", bufs=2, space="PSUM"))

        identb = const.tile([P, P], bf16)
        make_identity(nc, identb)

        dmaq = [nc.sync, nc.scalar]

        # ---- binarized weight tiles: sign(w) as fp8 [cin, pair, cout],
        # one shared f32 stage tile (reloaded per cout half) + fp8 sign
        # tiles per half so oc0's matmuls never wait on oc1's sign ----
        wsgn = [wpool.tile([P, 9, 2, P], fp8, name=f"wsgn{o}")
                for o in range(2)]

        def _load_w(oc):
            wt = wstage.tile([P, 9, 2, P], f32, name="wst")
            dmaq[oc].dma_start(
                out=wt,
                in_=w[:, :, :, P * oc : P * (oc + 1)].rearrange(
                    "ky kx (cc p) o -> p (ky kx) cc o", p=P
                ),
            )
            return wt

        def _sign(oc, wt):
            nc.scalar.sign(out=wsgn[oc], in_=wt)

        # HAM warmup: dummy matmuls keep the PE busy from t~0 while the
        # first image loads, so the p-state ramp reaches full clock before
        # the first conv matmuls arrive. Results are never read.
        wrm = psc.tile([P, RB, RW], f32, name="ps")
        for _ in range(warm):
            nc.tensor.matmul(
                wrm[:, :2, :], lhsT=identb, rhs=identb[:, : 2 * RW],
                start=True, stop=True,
            )

        def _alloc_image(img):
            st = {"img": img}
            st["hin"] = [hinp.tile([TB, 7, C], bf16, name=f"hin{q}")
                         for q in range(NCH)]
            st["xpb"] = [xbp.tile([P, 2, CB], bf16, name=f"xpb{q}")
                         for q in range(NCH)]
            st["xph"] = padp.tile([P, 2, FT], fp8, name="xph")
            st["xpl"] = padp.tile([P, 2, FT], fp8, name="xpl")
            # zero the SAME-padding borders (rows y=-1,56 and cols x=-1,56)
            # and the junk edge cells some shifted windows read
            for xp8 in (st["xph"], st["xpl"]):
                nc.vector.memset(xp8[:, :, 0:1], 0.0)
                nc.vector.memset(xp8[:, :, 1 + FL : FT], 0.0)
                xv = xp8[:, :, 1 : 1 + FL].rearrange(
                    "p j (r c) -> p j r c", c=RW
                )
                nc.vector.memset(xv[:, :, 0, :], 0.0)
                nc.vector.memset(xv[:, :, XW - 1, :], 0.0)
                nc.vector.memset(xv[:, :, 1 : XW - 1, 0], 0.0)
            return st

        def _load_chunk(st, q):
            # gpsimd SWDGE load, casting f32 -> bf16 in flight; both channel
            # halves in one DMA (innermost 512B out keeps full DMA rate)
            nc.gpsimd.dma_start(
                out=st["hin"][q],
                in_=x[st["img"], :, :].rearrange(
                    "(q b p) c -> q p b c", p=TB, q=NCH
                )[q],
            )

        def _emit_xbar(st, q, queue=None):
            # XBAR-transpose chunk q's blocks into channel-major bf16.
            # All 14 land on ONE queue: a tile written from two queues
            # serializes on cross-queue sync, one queue pipelines at 98ns.
            eng = dmaq[q % 2 if queue is None else queue]
            for b in range(7):
                for cc in range(2):
                    eng.dma_start(
                        out=st["xpb"][q][:, cc, TB * b : TB * (b + 1)],
                        in_=st["hin"][q][:, b, P * cc : P * (cc + 1)],
                        transpose=True,
                    )

        def _emit_hilo(st, q, eng):
            # derive hi = fp8(x), lo = fp8(x - hi) for chunk q (14 image
            # rows) into the flat-padded fp8 images
            r0 = 14 * q + 1
            for cc in range(2):
                bv = st["xpb"][q][:, cc, :].rearrange(
                    "p (r c) -> p r c", c=W
                )

                def _dst(xp8):
                    return xp8[:, cc, 1 : 1 + FL].rearrange(
                        "p (r c) -> p r c", c=RW
                    )[:, r0 : r0 + 14, 1 : 1 + W]

                eng.tensor_copy(out=_dst(st["xph"]), in_=bv)
                eng.tensor_sub(out=_dst(st["xpl"]), in0=bv,
                               in1=_dst(st["xph"]))

        def _emit_out_xbar(img, oc, ocmp):
            # flip the finished oc image back to pixel-major, upcast, store.
            # Runs entirely on SP/Act/DVE, overlapping the next oc's conv.
            ot = otp.tile([P, NOB, P], bf16, name="ot")
            onat = onp.tile([P, NOB, P], f32, name="onat")
            for j in range(NOB):
                dmaq[oc].dma_start(
                    out=ot[:, j, :],
                    in_=ocmp[:, OB * j : OB * (j + 1)],
                    transpose=True,
                )
            for bi, (j0, j1) in enumerate(((0, 12), (12, NOB))):
                nc.vector.tensor_copy(out=onat[:, j0:j1], in_=ot[:, j0:j1])
                jr = min(j1, NOB - 1)
                dmaq[(bi + oc) % 2].dma_start(
                    out=y[
                        img, OB * j0 : OB * jr, P * oc : P * (oc + 1)
                    ].rearrange("(b p) c -> p b c", p=OB),
                    in_=onat[:, j0:jr],
                )
            dmaq[oc % 2].dma_start(
                out=y[img, OB * (NOB - 1) : NPIX, P * oc : P * (oc + 1)
                      ].rearrange("(b p) c -> p b c", p=64),
                in_=onat[:64, NOB - 1],
            )

        def _emit_group_pe(img, oc, ocmp, t):
            # v1-style PE transpose path for the very last oc: group t's 4
            # TB-blocks (448 pixels) go psum->sbuf->store right away, so
            # the post-conv drain is one small group
            pt = psf.tile([P, 4, P], bf16, name="ptf")
            onatf = onf.tile([P, 4, P], f32, name="onatf")
            for bi in range(4):
                b = 4 * t + bi
                nc.tensor.transpose(
                    pt[:TB, bi, :], ocmp[:, TB * b : TB * (b + 1)], identb
                )
            nc.scalar.copy(out=onatf[:TB, :], in_=pt[:TB, :])
            for qi in range(2):
                dmaq[qi].dma_start(
                    out=y[
                        img,
                        TB * (4 * t + 2 * qi) : TB * (4 * t + 2 * qi + 2),
                        P * oc : P * (oc + 1),
                    ].rearrange("(b p) c -> p b c", p=TB),
                    in_=onatf[:TB, 2 * qi : 2 * qi + 2],
                )

        def _conv_image(st, nxt):
            # ---- conv: 18 accumulating DoubleRow matmuls per psum block
            # (hi/lo passes x 9 taps, all 256 cin per matmul). All other
            # work rides on DVE/Pool/SP/Act and never touches the PE queue
            # (except the last oc's tail transposes, when the PE is free).
            img = st["img"]
            combos = [
                (st["xph"], ky, kx) for ky in (1, 0, 2) for kx in range(3)
            ] + [
                (st["xpl"], ky, kx) for ky in (1, 0, 2) for kx in range(3)
            ]
            n_c = len(combos)

            for oc in range(2):
                fine = nxt is None and oc == 1
                ocmp = ocp.tile([P, OPIX], bf16, name="ocmp")
                if not fine:
                    nc.vector.memset(ocmp[:, NPIX:OPIX], 0.0)
                for t in range(NT):
                    ps = psc.tile([P, RB, RW], f32, name="ps")
                    for ci, (src8, ky, kx) in enumerate(combos):
                        dy = ky - 1
                        fs = (RB * t + dy + 1) * RW + kx
                        # skip the zero pad-row slice of the window for the
                        # edge taps (ci==0 is dy=0, so the start flag still
                        # clears the full region)
                        r0 = 1 if (t == 0 and dy < 0) else 0
                        r1 = RB - (1 if (t == NT - 1 and dy > 0) else 0)
                        nc.tensor.matmul(
                            ps[:, r0:r1, :],
                            lhsT=wsgn[oc][:, 3 * ky + kx, :, :],
                            rhs=src8[:, :, fs + r0 * RW : fs + r1 * RW],
                            start=(ci == 0),
                            stop=(ci == n_c - 1),
                            perf_mode=DR,
                        )
                        # spread next-image prep between this group's
                        # matmuls (non-PE queues, dependency-time order)
                        if nxt is not None and ci == 4:
                            if oc == 0:
                                if t == 0:
                                    for q in range(NCH):
                                        _load_chunk(nxt, q)
                                elif t in (1, 2, 3, 4):
                                    _emit_xbar(nxt, t - 1)  # queue t-1 %2
                            elif t in (0, 1, 2, 3):
                                _emit_hilo(nxt, t, nc.gpsimd)
                    # evict this group's rows to the bf16 compact image
                    nc.vector.tensor_copy(
                        out=ocmp[:, RB * W * t : RB * W * (t + 1)],
                        in_=ps[:, :, 1 : 1 + W],
                    )
                    if fine:
                        _emit_group_pe(img, oc, ocmp, t)
                if not fine:
                    _emit_out_xbar(img, oc, ocmp)

        def _images():
            # image 0 startup: chunk-0's load -> XBAR -> hi/lo chain is the
            # critical path to the first conv matmul. Weights ride the
            # HWDGE queues (parallel to the SWDGE chunk loads on Pool);
            # image-0 hi/lo runs on DVE (chunks 0-1) and Pool (2-3).
            st = _alloc_image(0)
            _load_chunk(st, 0)
            wt0 = _load_w(0)
            _load_chunk(st, 1)
            _load_chunk(st, 2)
            _load_chunk(st, 3)
            _sign(0, wt0)
            _emit_xbar(st, 0, queue=0)
            _emit_hilo(st, 0, nc.vector)
            _emit_xbar(st, 1, queue=0)
            _emit_hilo(st, 1, nc.vector)
            _emit_xbar(st, 2, queue=1)
            _emit_hilo(st, 2, nc.gpsimd)
            _emit_xbar(st, 3, queue=1)
            wt1 = _load_w(1)
            _sign(1, wt1)
            _emit_hilo(st, 3, nc.gpsimd)
            for img in range(ni):
                nxt = _alloc_image(img + 1) if img + 1 < ni else None
                _conv_image(st, nxt)
                st = nxt

        if loops == 1:
            _images()
        else:
            with tc.For_i(0, loops, 1):
                _images()
    nc.compile()
    return nc


def get_bass(ni=NI, loops=1):
    key = (ni, loops)
    if key not in _cache:
        _cache[key] = _build_bass(ni, loops)
    return _cache[key]


def run(inputs, kernel, trace=False, **kw):
    from concourse.bass_utils import run_bass_kernel_spmd

    nc = get_bass()
    xs = np.ascontiguousarray(inputs, dtype=np.float32).reshape(NTOT, NPIX, C)
    wf = np.ascontiguousarray(kernel, dtype=np.float32)
    in_maps = [
        {"x": xs[i * NI : (i + 1) * NI], "w": wf} for i in range(NCORES)
    ]
    res = run_bass_kernel_spmd(nc, in_maps, core_ids=list(range(NCORES)),
                               trace=trace, **kw)
    out = np.concatenate([r["y"] for r in res.results], axis=0)
    return out.reshape(NTOT, H, W, C), res


def kernel(**inputs):
    out, _ = run(inputs["inputs"], inputs["kernel"])
    return out
